# revision 6
# baseline (speedup 1.0000x reference)
"""Trainium2 Bass kernel: KmeansVectorQuantizer forward (vq_codebook).

Math (matching the jax reference bit-for-bit where it matters):
  latent = x.reshape(-1, 512)                      # [16384, 512]
  d[i,j] = fl(S_i + C_j - 2*M_ij)  with S_i=sum(latent_i^2), C_j=sum(W_j^2),
           M = latent @ W.T
  Because C_j (~2.5e-6) is strictly below half-ulp of S_i (~512), the
  reference's fp32 d collapses elementwise to fl(S_i - fl(2*M_ij)).
  indices = argmin_j d  (first index on ties)
  x_q = x + (W[indices] - x)        (straight-through, computed in this order)
  loss = 0.25 * mean((W[indices] - x)^2)

Device strategy (8 cores, data-parallel over tokens, 2048 tokens/core):
  - PE: v = (-2 x^T)^T @ W^T accumulated fp32 in PSUM, [128 tok, 512 code] tiles
  - ACT: d = Identity(v * 1 + S)  (PSUM->SBUF, per-partition bias) -> exact
    reference quantization of the distance
  - DVE: row min (tensor_reduce) + max_index(first occurrence of the min)
    == argmin with first-index tie-break, exactly the reference semantics
  - GPSIMD indirect DMA: gather W[winner] rows
  - DVE/ACT: diff, straight-through add, Square+accum for the loss partials
"""

import os
import sys

import numpy as np

for _p in ("/opt/trn_rl_repo", "/root/.axon_site/_ro/trn_rl_repo"):
    if os.path.isdir(_p) and _p not in sys.path:
        sys.path.insert(0, _p)

import concourse.bass as bass  # noqa: E402
import concourse.mybir as mybir  # noqa: E402
import concourse.tile as tile  # noqa: E402
from concourse import bacc  # noqa: E402
from concourse.bass_utils import run_bass_kernel_spmd  # noqa: E402

N_CORES = 8
NE = 8192           # codebook entries
E = 512             # embedding dim
TOK = 16384         # total tokens (32*512)
TPC = TOK // N_CORES  # tokens per core
P = 128             # partitions
TT = TPC // P       # token tiles per core (16)
KT = E // P         # contraction k-tiles (4)
CW = 512            # code-tile width (one PSUM bank of fp32)
CT = NE // CW       # code tiles (16)
BETA = 0.25

_BUILT = {}
LAST_RESULTS = None  # test harness introspection


def _build_program():
    f32 = mybir.dt.float32
    u32 = mybir.dt.uint32
    nc = bacc.Bacc(None, target_bir_lowering=False)

    xt2 = nc.dram_tensor("xt2", [E, TPC], f32, kind="ExternalInput")   # -2*x^T shard
    xin = nc.dram_tensor("xin", [TPC, E], f32, kind="ExternalInput")   # x shard
    wt = nc.dram_tensor("wt", [E, NE], f32, kind="ExternalInput")      # W^T
    wrow = nc.dram_tensor("wrow", [NE, E], f32, kind="ExternalInput")  # W rows (gather src)
    stl = nc.dram_tensor("stl", [P, TT], f32, kind="ExternalInput")    # S per (partition, tile)

    xq = nc.dram_tensor("xq", [TPC, E], f32, kind="ExternalOutput")
    idxo = nc.dram_tensor("idxo", [P, TT * 8], u32, kind="ExternalOutput")
    lsp = nc.dram_tensor("lsp", [P, TT], f32, kind="ExternalOutput")

    with tile.TileContext(nc) as tc:
        with (
            tc.tile_pool(name="wtp", bufs=1) as wtp,
            tc.tile_pool(name="xt2p", bufs=2) as xt2p,
            tc.tile_pool(name="dp", bufs=2) as dp,
            tc.tile_pool(name="psp", bufs=8, space="PSUM") as psp,
            tc.tile_pool(name="wkp", bufs=2) as wkp,
            tc.tile_pool(name="wk1", bufs=1) as wk1,
            tc.tile_pool(name="smp", bufs=1) as smp,
        ):
            # resident W^T k-tiles; chunked loads so matmuls start early
            wt_sb = []
            for k in range(KT):
                t_ = wtp.tile([P, NE], f32, tag=f"wt{k}")
                wt_sb.append(t_)
                for t in range(CT):
                    nc.sync.dma_start(
                        out=t_[:, t * CW:(t + 1) * CW],
                        in_=wt[k * P:(k + 1) * P, t * CW:(t + 1) * CW],
                    )

            s_sb = smp.tile([P, TT], f32, tag="s")
            nc.sync.dma_start(out=s_sb[:], in_=stl[:])
            loss_sb = smp.tile([P, TT], f32, tag="loss")
            idxw = smp.tile([P, TT * 8], u32, tag="idxw")

            for i in range(TT):
                # this token-tile's -2x^T, packed [p, k, m]
                xt = xt2p.tile([P, KT, P], f32, tag="xt2")
                nc.sync.dma_start(
                    out=xt[:],
                    in_=xt2[:, i * P:(i + 1) * P].rearrange("(k p) m -> p k m", p=P),
                )
                # natural x rows for this tile
                xnat = wkp.tile([P, E], f32, tag="xnat")
                nc.sync.dma_start(out=xnat[:], in_=xin[i * P:(i + 1) * P, :])

                drow = dp.tile([P, NE], f32, tag="d")
                for t in range(CT):
                    pt = psp.tile([P, CW], f32, tag="ps")
                    for k in range(KT):
                        nc.tensor.matmul(
                            out=pt[:],
                            lhsT=xt[:, k, :],
                            rhs=wt_sb[k][:, t * CW:(t + 1) * CW],
                            start=(k == 0),
                            stop=(k == KT - 1),
                        )
                    # d = fl(v + S_i): the reference's exact fp32 quantization
                    nc.scalar.activation(
                        out=drow[:, t * CW:(t + 1) * CW],
                        in_=pt[:],
                        func=mybir.ActivationFunctionType.Identity,
                        bias=s_sb[:, i:i + 1],
                        scale=1.0,
                    )

                dmin = wk1.tile([P, 1], f32, tag="dmin")
                nc.vector.tensor_reduce(
                    out=dmin[:], in_=drow[:],
                    axis=mybir.AxisListType.X, op=mybir.AluOpType.min,
                )
                dmb = wk1.tile([P, 8], f32, tag="dmb")
                nc.vector.tensor_copy(out=dmb[:], in_=dmin[:].to_broadcast([P, 8]))
                # first occurrence of the min == reference argmin tie-break
                nc.vector.max_index(
                    out=idxw[:, i * 8:(i + 1) * 8], in_max=dmb[:], in_values=drow[:],
                )

                wq = wkp.tile([P, E], f32, tag="wq")
                nc.gpsimd.indirect_dma_start(
                    out=wq[:],
                    out_offset=None,
                    in_=wrow[:],
                    in_offset=bass.IndirectOffsetOnAxis(
                        ap=idxw[:, i * 8:i * 8 + 1], axis=0,
                    ),
                )

                diff = wk1.tile([P, E], f32, tag="diff")
                nc.vector.tensor_tensor(
                    out=diff[:], in0=wq[:], in1=xnat[:], op=mybir.AluOpType.subtract,
                )
                # straight-through: x + diff, in place over x
                nc.vector.tensor_tensor(
                    out=xnat[:], in0=xnat[:], in1=diff[:], op=mybir.AluOpType.add,
                )
                nc.sync.dma_start(out=xq[i * P:(i + 1) * P, :], in_=xnat[:])
                # loss partial: sum(diff^2) per partition (ACT Square + accum)
                nc.scalar.activation(
                    out=diff[:],
                    in_=diff[:],
                    func=mybir.ActivationFunctionType.Square,
                    accum_out=loss_sb[:, i:i + 1],
                )

            nc.sync.dma_start(out=lsp[:], in_=loss_sb[:])
            nc.sync.dma_start(out=idxo[:], in_=idxw[:])

    nc.finalize()
    return nc


def _get_program():
    if "nc" not in _BUILT:
        _BUILT["nc"] = _build_program()
    return _BUILT["nc"]


def kernel(x, label, idx, W):
    global LAST_RESULTS
    x = np.asarray(x, dtype=np.float32)
    W = np.ascontiguousarray(np.asarray(W, dtype=np.float32))

    lat = np.ascontiguousarray(x.reshape(TOK, E))
    xt2_full = np.ascontiguousarray((-2.0 * lat).T)    # [E, TOK]
    wt_full = np.ascontiguousarray(W.T)                # [E, NE]
    s_full = np.sum(lat * lat, axis=1, dtype=np.float32)  # [TOK]

    in_maps = []
    for c in range(N_CORES):
        sl = slice(c * TPC, (c + 1) * TPC)
        in_maps.append({
            "xt2": np.ascontiguousarray(xt2_full[:, sl]),
            "xin": lat[sl],
            "wt": wt_full,
            "wrow": W,
            "stl": np.ascontiguousarray(s_full[sl].reshape(TT, P).T),
        })

    nc = _get_program()
    res = run_bass_kernel_spmd(nc, in_maps, core_ids=list(range(N_CORES)))
    LAST_RESULTS = res
    outs = res.results

    xq = np.concatenate([o["xq"] for o in outs], axis=0).reshape(x.shape)
    idx_parts = [o["idxo"][:, ::8].T.reshape(TPC) for o in outs]
    indices = np.concatenate(idx_parts).astype(np.int32).reshape(x.shape[:2])
    loss_sum = float(sum(o["lsp"].astype(np.float64).sum() for o in outs))
    loss = np.float32(BETA * (loss_sum / (TOK * E)))
    return xq, loss, indices
